# revision 6
# baseline (speedup 1.0000x reference)
"""Deformable conv block (offset conv -> bilinear deform depthwise -> pointwise)
on 8 Trainium2 NeuronCores, data-parallel over batch (2 per core).

v3 (v2 at 1.37ms, v1 at 2.17ms):
  - gather: ONE idx per (pixel, tap) fetching all 4 bilinear corners from a
    host-built row-pair-duplicated bf16 image; w_dw folded in per tap
  - corner combine on PE via per-pixel diagonal stationaries, PSUM-accumulated
    over all 36 (tap, corner) pairs; 1/4 of diag builds on the Scalar engine
  - idx 16-partition wrap via contiguous DRAM dump + 8 strided replicate
    reads (576B runs) instead of 2-byte scattered writes
  - both batches' conv/field/idx phase emitted before the combines so the
    Pool engine (gather desc-gen, the ~660us/core floor) never idles
"""

import numpy as np
import ml_dtypes

import concourse.bass as bass
import concourse.bacc as bacc
import concourse.tile as tile
from concourse import mybir
from concourse.bass_utils import run_bass_kernel_spmd
from concourse.masks import make_identity

F32 = mybir.dt.float32
BF16 = mybir.dt.bfloat16
I16 = mybir.dt.int16
AF = mybir.AluOpType

B, C, CO, H, W = 16, 192, 384, 64, 64
HW = H * W
K2 = 9
PADG = 4
WG = W + 2 * PADG          # 72 gather-image rows/cols
NEG = WG * WG              # 5184 gather elems per image
ESZ = 4 * C                # 768 bf16 values per gather elem (2px x 2rows x C)
NCORES = 8
BPC = B // NCORES          # 2
WC = W + 2                 # 66 conv-pad
NPX = 32                   # pixel groups of 128
NCH = 8                    # gather chunks per batch (512 px each)
GPC = NPX // NCH           # 4 pixel-groups per chunk
NI = 128 * GPC             # 512 idxs per gather

_cache = {}


def _build():
    if "nc" in _cache:
        return _cache["nc"]
    nc = bacc.Bacc("TRN2", target_bir_lowering=False, debug=False)

    xc0 = nc.dram_tensor("xc0", [BPC, 128, WC, WC], BF16, kind="ExternalInput")
    xc1 = nc.dram_tensor("xc1", [BPC, 64, WC, WC], BF16, kind="ExternalInput")
    xg = nc.dram_tensor("xg", [BPC, K2, NEG, ESZ], BF16, kind="ExternalInput")
    woff0 = nc.dram_tensor("woff0", [128, 9, 18], BF16, kind="ExternalInput")
    woff1 = nc.dram_tensor("woff1", [64, 9, 18], BF16, kind="ExternalInput")
    cstT = nc.dram_tensor("cstT", [128, NPX, 18], F32, kind="ExternalInput")
    wpw0 = nc.dram_tensor("wpw0", [128, CO], BF16, kind="ExternalInput")
    wpw1 = nc.dram_tensor("wpw1", [64, CO], BF16, kind="ExternalInput")
    out_d = nc.dram_tensor("out", [BPC, CO, HW], BF16, kind="ExternalOutput")
    idx_dram = nc.dram_tensor("idx_scratch", [BPC, 128, K2 * NPX], I16)

    with tile.TileContext(nc) as tc:
        import contextlib
        with contextlib.ExitStack() as ctx:
            singles = ctx.enter_context(tc.tile_pool(name="singles", bufs=1))
            work = ctx.enter_context(tc.tile_pool(name="work", bufs=2))
            fbuf = ctx.enter_context(tc.tile_pool(name="fbuf", bufs=2))
            gbuf = ctx.enter_context(tc.tile_pool(name="gbuf", bufs=3))
            dbuf = ctx.enter_context(tc.tile_pool(name="dbuf", bufs=16))
            tbuf = ctx.enter_context(tc.tile_pool(name="tbuf", bufs=2))
            obuf = ctx.enter_context(tc.tile_pool(name="obuf", bufs=3))
            ps_acc = ctx.enter_context(tc.tile_pool(name="ps_acc", bufs=1, space="PSUM"))
            ps_mm = ctx.enter_context(tc.tile_pool(name="ps_mm", bufs=2, space="PSUM"))

            ident = singles.tile([128, 128], F32)
            make_identity(nc, ident[:, :])
            identb = singles.tile([128, 128], BF16)
            make_identity(nc, identb[:, :])
            s_w0 = singles.tile([128, 9, 18], BF16, tag="sw0")
            nc.sync.dma_start(out=s_w0[:, :, :], in_=woff0[:, :, :])
            s_w1 = singles.tile([64, 9, 18], BF16, tag="sw1")
            nc.sync.dma_start(out=s_w1[:, :, :], in_=woff1[:, :, :])
            s_cT = singles.tile([128, NPX, 18], F32, tag="scT")
            nc.sync.dma_start(out=s_cT[:, :, :], in_=cstT[:, :, :])
            s_p0 = singles.tile([128, CO], BF16, tag="sp0")
            nc.sync.dma_start(out=s_p0[:, :], in_=wpw0[:, :])
            s_p1 = singles.tile([64, CO], BF16, tag="sp1")
            nc.sync.dma_start(out=s_p1[:, :], in_=wpw1[:, :])

            # ---------------- phase A (both batches): conv + field + idx ---
            wgt_b = []
            idxw_b = []
            for b in range(BPC):
                s_x0 = work.tile([128, WC, WC], BF16, tag="x0")
                nc.sync.dma_start(out=s_x0[:, :, :], in_=xc0[b])
                s_x1 = work.tile([64, WC, WC], BF16, tag="x1")
                nc.sync.dma_start(out=s_x1[:, :, :], in_=xc1[b])

                # offset conv (bf16, PSUM f32), transpose per 512-px chunk
                offT = work.tile([128, NPX, 18], F32, tag="offT")
                for q in range(8):
                    pch = ps_mm.tile([128, 512], F32, tag="mm")
                    mm = 0
                    for s in range(9):
                        dy, dx = s // 3, s % 3
                        for src, wt in ((s_x0, s_w0), (s_x1, s_w1)):
                            nc.tensor.matmul(
                                pch[0:18, :],
                                wt[:, s, :],
                                src[:, 8 * q + dy:8 * q + dy + 8, dx:dx + 64],
                                start=(mm == 0),
                                stop=(mm == 17),
                            )
                            mm += 1
                    off_q = work.tile([18, 512], F32, tag="offq")
                    nc.vector.tensor_copy(off_q[:, :], pch[0:18, :])
                    for u in range(4):
                        ptr = ps_mm.tile([128, 512], F32, tag="mm")
                        nc.tensor.transpose(
                            ptr[:, 0:18], off_q[:, 128 * u:128 * (u + 1)],
                            ident[:18, :18]
                        )
                        nc.vector.tensor_copy(offT[:, 4 * q + u, :], ptr[:, 0:18])

                # field: pos -> floor/frac -> idx + bilinear weights
                pos = fbuf.tile([128, NPX, 18], F32, tag="pos")
                nc.vector.tensor_tensor(pos[:, :, :], offT[:, :, :], s_cT[:, :, :], AF.add)
                nc.vector.tensor_scalar(pos[:, :, :], pos[:, :, :], 130.5, 60.5, AF.min, AF.max)
                fl = fbuf.tile([128, NPX, 18], F32, tag="fl")
                nc.vector.tensor_scalar(fl[:, :, :], pos[:, :, :], 8388608.0, -8388608.0, AF.add, AF.add)
                frac = fbuf.tile([128, NPX, 18], F32, tag="frac")
                nc.vector.tensor_tensor(frac[:, :, :], fl[:, :, :], pos[:, :, :], AF.is_gt)
                nc.vector.tensor_tensor(fl[:, :, :], fl[:, :, :], frac[:, :, :], AF.subtract)
                nc.vector.tensor_tensor(frac[:, :, :], pos[:, :, :], fl[:, :, :], AF.subtract)
                g1 = fbuf.tile([128, NPX, 18], F32, tag="g1")
                nc.vector.tensor_scalar(g1[:, :, :], frac[:, :, :], -1.0, 1.0, AF.mult, AF.add)
                # corner order in gather elem: (y0,x0), (y1,x0), (y0,x1), (y1,x1)
                wgt = fbuf.tile([128, 4, NPX, K2], F32, tag="wgt")
                nc.vector.tensor_tensor(wgt[:, 0], g1[:, :, 0:9], g1[:, :, 9:18], AF.mult)
                nc.vector.tensor_tensor(wgt[:, 1], frac[:, :, 0:9], g1[:, :, 9:18], AF.mult)
                nc.vector.tensor_tensor(wgt[:, 2], g1[:, :, 0:9], frac[:, :, 9:18], AF.mult)
                nc.vector.tensor_tensor(wgt[:, 3], frac[:, :, 0:9], frac[:, :, 9:18], AF.mult)
                wgt_b.append(wgt)

                idxf = fbuf.tile([128, K2, NPX], F32, tag="idxf")
                _if = idxf[:, :, :]
                idxf_v = bass.AP(tensor=_if.tensor, offset=_if.offset,
                                 ap=[_if.ap[0], [1, NPX], [NPX, K2]])
                nc.vector.scalar_tensor_tensor(
                    idxf_v, fl[:, :, 0:9], 72.0, fl[:, :, 9:18], AF.mult, AF.add
                )
                idx16 = fbuf.tile([128, K2, NPX], I16, tag="idx16")
                nc.vector.tensor_scalar(idx16[:, :, :], idxf[:, :, :], -4380.0, None, AF.add)

                # idx wrap: contiguous dump, then 8 strided replicate reads.
                # dram[p, k*32 + t] = idx of pixel 128t + p (k-major, contiguous)
                base = idx_dram[b]
                dump_out = bass.AP(
                    tensor=base.tensor, offset=base.offset,
                    ap=[[288, 128], [1, 288]],
                )
                nc.sync.dma_start(out=dump_out, in_=idx16[:, :, :])
                # idxw[16a+r, k, ch, m] = idx(pixel 512ch+16m+r)
                #   = dram[16*(m%8)+r, k*32 + 4*ch + m//8]
                idxw = fbuf.tile([128, K2, NCH, 32], I16, tag="idxw")
                for pg in range(8):
                    dst = bass.AP(
                        tensor=idxw.tensor, offset=idxw.offset + pg,
                        ap=[idxw.ap[0], [8, 288]],
                    )
                    rep = bass.AP(
                        tensor=base.tensor, offset=base.offset + 4608 * pg,
                        ap=[[0, 8], [288, 16], [1, 288]],
                    )
                    nc.sync.dma_start(out=dst, in_=rep)
                idxw_b.append(idxw)

            # ---------------- phase B (both batches): gather + combine -----
            for b in range(BPC):
                wgt = wgt_b[b]
                idxw = idxw_b[b]
                xg_b = xg[b]
                for ch in range(NCH):
                    acc_ts = [ps_acc.tile([128, 512], F32, tag=f"acc{i}",
                                          name=f"acc_{b}_{ch}_{i}")
                              for i in range(GPC)]
                    for k in range(K2):
                        g = gbuf.tile([128, GPC, ESZ], BF16, tag="g")
                        xgk = xg_b[k]
                        src = bass.AP(
                            tensor=xgk.tensor,
                            offset=xgk.offset,
                            ap=[[ESZ, NEG], [1, ESZ]],
                        )
                        nc.gpsimd.dma_gather(
                            out_ap=g[:, :, :],
                            in_ap=src,
                            idxs_ap=idxw[:, k, ch, :],
                            num_idxs=NI,
                            num_idxs_reg=NI,
                            elem_size=ESZ,
                            elem_step=ESZ,
                        )
                        for c in range(GPC):
                            t = GPC * ch + c
                            for j in range(4):
                                diag = dbuf.tile([128, 128], BF16, tag="diag")
                                if j == 2:
                                    nc.scalar.mul(
                                        diag[:, :], identb[:, :],
                                        wgt[:, j, t, k:k + 1],
                                    )
                                else:
                                    nc.vector.tensor_scalar(
                                        diag[:, :], identb[:, :],
                                        wgt[:, j, t, k:k + 1], None, AF.mult,
                                    )
                                nc.tensor.matmul(
                                    acc_ts[c][:, 0:C],
                                    diag[:, :],
                                    g[:, c, C * j:C * (j + 1)],
                                    start=(k == 0 and j == 0),
                                    stop=(k == K2 - 1 and j == 3),
                                )

                    # psum -> sbuf (Scalar engine), transpose to c-major (bf16)
                    acc_sb = tbuf.tile([128, GPC, C], BF16, tag="accsb")
                    for c in range(GPC):
                        nc.scalar.copy(acc_sb[:, c, :], acc_ts[c][:, 0:C])
                    dwT0 = tbuf.tile([128, 512], BF16, tag="dwT0")
                    dwT1 = tbuf.tile([64, 512], BF16, tag="dwT1")
                    for c in range(GPC):
                        pt = ps_mm.tile([128, 256], BF16, tag="mmb")
                        nc.tensor.transpose(pt[:, 0:128], acc_sb[:, c, 0:128], identb[:, :])
                        nc.tensor.transpose(pt[0:64, 128:256], acc_sb[:, c, 128:192], identb[:, :])
                        nc.vector.tensor_copy(dwT0[:, 128 * c:128 * (c + 1)], pt[:, 0:128])
                        nc.vector.tensor_copy(dwT1[:, 128 * c:128 * (c + 1)], pt[0:64, 128:256])

                    # pointwise
                    for o in range(3):
                        ppw = ps_mm.tile([128, 512], F32, tag="mm")
                        nc.tensor.matmul(
                            ppw[:, :], s_p0[:, 128 * o:128 * (o + 1)], dwT0[:, :],
                            start=True, stop=False,
                        )
                        nc.tensor.matmul(
                            ppw[:, :], s_p1[:, 128 * o:128 * (o + 1)], dwT1[:, :],
                            start=False, stop=True,
                        )
                        osb = obuf.tile([128, 512], BF16, tag="osb")
                        nc.scalar.copy(osb[:, :], ppw[:, :])
                        nc.sync.dma_start(
                            out=out_d[b, 128 * o:128 * (o + 1), 512 * ch:512 * (ch + 1)],
                            in_=osb[:, :],
                        )

    nc.compile()
    _cache["nc"] = nc
    return nc


def _host_prep(x, w_off, b_off, w_dw, w_pw):
    K = 3
    bf = ml_dtypes.bfloat16
    # conv input, zero-padded by 1, c-major
    xcp = np.zeros((B, C, WC, WC), bf)
    xcp[:, :, 1:65, 1:65] = x
    # per-tap dw-scaled gather images: row-pair + x-pair duplicated, px-major
    # xg[b, k, yy*72+xx, (dx*2+r)*C + c] = x[b, c, yy-4+r, xx-4+dx] * dw[c, k]
    wdw = w_dw.reshape(C, K2)
    xg = np.empty((B, K2, NEG, ESZ), bf)
    P2 = np.zeros((B, WG + 1, WG + 1, C), np.float32)
    P2[:, PADG:PADG + H, PADG:PADG + W, :] = np.transpose(x, (0, 2, 3, 1))
    for k in range(K2):
        P2k = (P2 * wdw[None, None, None, :, k]).astype(bf)
        v = xg[:, k].reshape(B, WG, WG, 2, 2, C)
        for dx in range(2):
            for r in range(2):
                v[:, :, :, dx, r, :] = P2k[:, r:r + WG, dx:dx + WG, :]

    # offset conv stationaries, out channels reordered to [y taps | x taps]
    perm = [2 * k for k in range(K2)] + [2 * k + 1 for k in range(K2)]
    wo = np.empty((9, C, 18), np.float32)
    for s in range(9):
        dy, dx = s // 3, s % 3
        wo[s] = w_off[perm, :, dy, dx].T  # [C, 18]
    wo = wo.transpose(1, 0, 2).astype(bf)  # [C, 9, 18]

    # px-major const: pos64 = off + base + ki/kj - 1 + b_off + 64
    i = np.arange(HW)
    hh, ww = i // W, i % W
    cst = np.empty((HW, 18), np.float32)
    for k in range(K2):
        ki, kj = k // K, k % K
        cst[:, k] = hh - 1 + ki + b_off[2 * k] + 64.0
        cst[:, 9 + k] = ww - 1 + kj + b_off[2 * k + 1] + 64.0
    cstT = cst.reshape(NPX, 128, 18).transpose(1, 0, 2).copy()  # [128, NPX, 18]

    wpwT = w_pw.T.astype(bf)  # [C, CO]

    shared = {
        "woff0": wo[:128].copy(),
        "woff1": wo[128:].copy(),
        "cstT": cstT,
        "wpw0": wpwT[:128].copy(),
        "wpw1": wpwT[128:].copy(),
    }
    in_maps = []
    for cid in range(NCORES):
        bs = slice(cid * BPC, (cid + 1) * BPC)
        m = dict(shared)
        m["xc0"] = xcp[bs, :128]
        m["xc1"] = xcp[bs, 128:]
        m["xg"] = xg[bs]
        in_maps.append(m)
    return in_maps


def kernel(x, w_off, b_off, w_dw, w_pw, _trace=False):
    x = np.asarray(x, np.float32)
    w_off = np.asarray(w_off, np.float32)
    b_off = np.asarray(b_off, np.float32)
    w_dw = np.asarray(w_dw, np.float32)
    w_pw = np.asarray(w_pw, np.float32)

    nc = _build()
    in_maps = _host_prep(x, w_off, b_off, w_dw, w_pw)
    res = run_bass_kernel_spmd(nc, in_maps, core_ids=list(range(NCORES)), trace=_trace)
    out = np.concatenate([np.asarray(r["out"], np.float32) for r in res.results], axis=0)
    if _trace:
        kernel.last_exec_ns = res.exec_time_ns
    return out.reshape(B, CO, H, W)


# revision 7
# speedup vs baseline: 3.2513x; 3.2513x over previous
"""Deformable conv block (offset conv -> bilinear deform depthwise -> pointwise)
on 8 Trainium2 NeuronCores, data-parallel over batch (2 per core).

v3 (v2 at 1.37ms, v1 at 2.17ms):
  - gather: ONE idx per (pixel, tap) fetching all 4 bilinear corners from a
    host-built row-pair-duplicated bf16 image; w_dw folded in per tap
  - corner combine on PE via per-pixel diagonal stationaries, PSUM-accumulated
    over all 36 (tap, corner) pairs; 1/4 of diag builds on the Scalar engine
  - idx 16-partition wrap via contiguous DRAM dump + 8 strided replicate
    reads (576B runs) instead of 2-byte scattered writes
  - both batches' conv/field/idx phase emitted before the combines so the
    Pool engine (gather desc-gen, the ~660us/core floor) never idles
"""

import numpy as np
import ml_dtypes

import concourse.bass as bass
import concourse.bacc as bacc
import concourse.tile as tile
from concourse import mybir
from concourse.bass_utils import run_bass_kernel_spmd
from concourse.masks import make_identity

F32 = mybir.dt.float32
BF16 = mybir.dt.bfloat16
I16 = mybir.dt.int16
AF = mybir.AluOpType

B, C, CO, H, W = 16, 192, 384, 64, 64
HW = H * W
K2 = 9
PADG = 4
WG = W + 2 * PADG          # 72 gather-image rows/cols
NEG = WG * WG              # 5184 gather elems per image
ESZ = 4 * C                # 768 bf16 values per gather elem (2px x 2rows x C)
NCORES = 8
BPC = B // NCORES          # 2
WC = W + 2                 # 66 conv-pad
NPX = 32                   # pixel groups of 128
NCH = 8                    # gather chunks per batch (512 px each)
GPC = NPX // NCH           # 4 pixel-groups per chunk
NI = 128 * GPC             # 512 idxs per gather

_cache = {}


def _build():
    if "nc" in _cache:
        return _cache["nc"]
    nc = bacc.Bacc("TRN2", target_bir_lowering=False, debug=False)

    xc0 = nc.dram_tensor("xc0", [BPC, 128, WC, WC], BF16, kind="ExternalInput")
    xc1 = nc.dram_tensor("xc1", [BPC, 64, WC, WC], BF16, kind="ExternalInput")
    xg = nc.dram_tensor("xg", [BPC, K2, NEG, ESZ], BF16, kind="ExternalInput")
    woff0 = nc.dram_tensor("woff0", [128, 9, 18], BF16, kind="ExternalInput")
    woff1 = nc.dram_tensor("woff1", [64, 9, 18], BF16, kind="ExternalInput")
    cstT = nc.dram_tensor("cstT", [128, NPX, 18], F32, kind="ExternalInput")
    wpw0 = nc.dram_tensor("wpw0", [128, CO], BF16, kind="ExternalInput")
    wpw1 = nc.dram_tensor("wpw1", [64, CO], BF16, kind="ExternalInput")
    out_d = nc.dram_tensor("out", [BPC, CO, HW], BF16, kind="ExternalOutput")
    idx_dram = nc.dram_tensor("idx_scratch", [BPC, 128, K2 * NPX], I16)

    with tile.TileContext(nc) as tc:
        import contextlib
        with contextlib.ExitStack() as ctx:
            singles = ctx.enter_context(tc.tile_pool(name="singles", bufs=1))
            work = ctx.enter_context(tc.tile_pool(name="work", bufs=2))
            fbuf = ctx.enter_context(tc.tile_pool(name="fbuf", bufs=2))
            gbuf = ctx.enter_context(tc.tile_pool(name="gbuf", bufs=3))
            dbuf = ctx.enter_context(tc.tile_pool(name="dbuf", bufs=16))
            tbuf = ctx.enter_context(tc.tile_pool(name="tbuf", bufs=2))
            obuf = ctx.enter_context(tc.tile_pool(name="obuf", bufs=3))
            ps_acc = ctx.enter_context(tc.tile_pool(name="ps_acc", bufs=1, space="PSUM"))
            ps_mm = ctx.enter_context(tc.tile_pool(name="ps_mm", bufs=2, space="PSUM"))

            ident = singles.tile([128, 128], F32)
            make_identity(nc, ident[:, :])
            identb = singles.tile([128, 128], BF16)
            make_identity(nc, identb[:, :])
            s_w0 = singles.tile([128, 9, 18], BF16, tag="sw0")
            nc.sync.dma_start(out=s_w0[:, :, :], in_=woff0[:, :, :])
            s_w1 = singles.tile([64, 9, 18], BF16, tag="sw1")
            nc.sync.dma_start(out=s_w1[:, :, :], in_=woff1[:, :, :])
            s_cT = singles.tile([128, NPX, 18], F32, tag="scT")
            nc.sync.dma_start(out=s_cT[:, :, :], in_=cstT[:, :, :])
            s_p0 = singles.tile([128, CO], BF16, tag="sp0")
            nc.sync.dma_start(out=s_p0[:, :], in_=wpw0[:, :])
            s_p1 = singles.tile([64, CO], BF16, tag="sp1")
            nc.sync.dma_start(out=s_p1[:, :], in_=wpw1[:, :])

            # ---------------- phase A (both batches): conv + field + idx ---
            wgt_b = []
            idxw_b = []
            for b in range(BPC):
                s_x0 = work.tile([128, WC, WC], BF16, tag="x0")
                nc.sync.dma_start(out=s_x0[:, :, :], in_=xc0[b])
                s_x1 = work.tile([64, WC, WC], BF16, tag="x1")
                nc.sync.dma_start(out=s_x1[:, :, :], in_=xc1[b])

                # offset conv (bf16, PSUM f32), transpose per 512-px chunk
                offT = work.tile([128, NPX, 18], F32, tag="offT")
                for q in range(8):
                    pch = ps_mm.tile([128, 512], F32, tag="mm")
                    mm = 0
                    for s in range(9):
                        dy, dx = s // 3, s % 3
                        for src, wt in ((s_x0, s_w0), (s_x1, s_w1)):
                            nc.tensor.matmul(
                                pch[0:18, :],
                                wt[:, s, :],
                                src[:, 8 * q + dy:8 * q + dy + 8, dx:dx + 64],
                                start=(mm == 0),
                                stop=(mm == 17),
                            )
                            mm += 1
                    off_q = work.tile([18, 512], F32, tag="offq")
                    nc.vector.tensor_copy(off_q[:, :], pch[0:18, :])
                    for u in range(4):
                        ptr = ps_mm.tile([128, 512], F32, tag="mm")
                        nc.tensor.transpose(
                            ptr[:, 0:18], off_q[:, 128 * u:128 * (u + 1)],
                            ident[:18, :18]
                        )
                        nc.vector.tensor_copy(offT[:, 4 * q + u, :], ptr[:, 0:18])

                # field: pos -> floor/frac -> idx + bilinear weights
                pos = fbuf.tile([128, NPX, 18], F32, tag="pos")
                nc.vector.tensor_tensor(pos[:, :, :], offT[:, :, :], s_cT[:, :, :], AF.add)
                nc.vector.tensor_scalar(pos[:, :, :], pos[:, :, :], 130.5, 60.5, AF.min, AF.max)
                fl = fbuf.tile([128, NPX, 18], F32, tag="fl")
                nc.vector.tensor_scalar(fl[:, :, :], pos[:, :, :], 8388608.0, -8388608.0, AF.add, AF.add)
                frac = fbuf.tile([128, NPX, 18], F32, tag="frac")
                nc.vector.tensor_tensor(frac[:, :, :], fl[:, :, :], pos[:, :, :], AF.is_gt)
                nc.vector.tensor_tensor(fl[:, :, :], fl[:, :, :], frac[:, :, :], AF.subtract)
                nc.vector.tensor_tensor(frac[:, :, :], pos[:, :, :], fl[:, :, :], AF.subtract)
                g1 = fbuf.tile([128, NPX, 18], F32, tag="g1")
                nc.vector.tensor_scalar(g1[:, :, :], frac[:, :, :], -1.0, 1.0, AF.mult, AF.add)
                # corner order in gather elem: (y0,x0), (y1,x0), (y0,x1), (y1,x1)
                wgt = fbuf.tile([128, 4, NPX, K2], F32, tag="wgt")
                nc.vector.tensor_tensor(wgt[:, 0], g1[:, :, 0:9], g1[:, :, 9:18], AF.mult)
                nc.vector.tensor_tensor(wgt[:, 1], frac[:, :, 0:9], g1[:, :, 9:18], AF.mult)
                nc.vector.tensor_tensor(wgt[:, 2], g1[:, :, 0:9], frac[:, :, 9:18], AF.mult)
                nc.vector.tensor_tensor(wgt[:, 3], frac[:, :, 0:9], frac[:, :, 9:18], AF.mult)
                wgt_b.append(wgt)

                idxf = fbuf.tile([128, K2, NPX], F32, tag="idxf")
                _if = idxf[:, :, :]
                idxf_v = bass.AP(tensor=_if.tensor, offset=_if.offset,
                                 ap=[_if.ap[0], [1, NPX], [NPX, K2]])
                nc.vector.scalar_tensor_tensor(
                    idxf_v, fl[:, :, 0:9], 72.0, fl[:, :, 9:18], AF.mult, AF.add
                )
                idx16 = fbuf.tile([128, K2, NPX], I16, tag="idx16")
                nc.vector.tensor_scalar(idx16[:, :, :], idxf[:, :, :], -4380.0, None, AF.add)

                # idx wrap: contiguous dump, then 8 strided replicate reads.
                # dram[p, k*32 + t] = idx of pixel 128t + p (k-major, contiguous)
                base = idx_dram[b]
                dump_out = bass.AP(
                    tensor=base.tensor, offset=base.offset,
                    ap=[[288, 128], [1, 288]],
                )
                nc.sync.dma_start(out=dump_out, in_=idx16[:, :, :])
                # idxw[16a+r, k, ch, m] = idx(pixel 512ch+16m+r)
                #   = dram[16*(m%8)+r, k*32 + 4*ch + m//8]
                # DMA both-sides-contiguous into tmp, then the stride-8
                # m-interleave is a cheap on-chip DVE copy.
                tmpw = fbuf.tile([128, 8, 288], I16, tag="tmpw")
                for pg in range(8):
                    rep = bass.AP(
                        tensor=base.tensor, offset=base.offset + 4608 * pg,
                        ap=[[0, 8], [288, 16], [1, 288]],
                    )
                    nc.sync.dma_start(out=tmpw[:, pg, :], in_=rep)
                idxw = fbuf.tile([128, K2, NCH, 32], I16, tag="idxw")
                iv = idxw[:, :, :, :]
                idxw_il = bass.AP(
                    tensor=iv.tensor, offset=iv.offset,
                    ap=[iv.ap[0], [1, 8], [8, 288]],
                )
                nc.vector.tensor_copy(idxw_il, tmpw[:, :, :])
                idxw_b.append(idxw)

            # ---------------- phase B (both batches): gather + combine -----
            for b in range(BPC):
                wgt = wgt_b[b]
                idxw = idxw_b[b]
                xg_b = xg[b]
                for ch in range(NCH):
                    acc_ts = [ps_acc.tile([128, 512], F32, tag=f"acc{i}",
                                          name=f"acc_{b}_{ch}_{i}")
                              for i in range(GPC)]
                    for k in range(K2):
                        g = gbuf.tile([128, GPC, ESZ], BF16, tag="g")
                        xgk = xg_b[k]
                        src = bass.AP(
                            tensor=xgk.tensor,
                            offset=xgk.offset,
                            ap=[[ESZ, NEG], [1, ESZ]],
                        )
                        nc.gpsimd.dma_gather(
                            out_ap=g[:, :, :],
                            in_ap=src,
                            idxs_ap=idxw[:, k, ch, :],
                            num_idxs=NI,
                            num_idxs_reg=NI,
                            elem_size=ESZ,
                            elem_step=ESZ,
                        )
                        for c in range(GPC):
                            t = GPC * ch + c
                            for j in range(4):
                                diag = dbuf.tile([128, 128], BF16, tag="diag")
                                if j == 2:
                                    nc.scalar.mul(
                                        diag[:, :], identb[:, :],
                                        wgt[:, j, t, k:k + 1],
                                    )
                                else:
                                    nc.vector.tensor_scalar(
                                        diag[:, :], identb[:, :],
                                        wgt[:, j, t, k:k + 1], None, AF.mult,
                                    )
                                nc.tensor.matmul(
                                    acc_ts[c][:, 0:C],
                                    diag[:, :],
                                    g[:, c, C * j:C * (j + 1)],
                                    start=(k == 0 and j == 0),
                                    stop=(k == K2 - 1 and j == 3),
                                )

                    # psum -> sbuf (Scalar engine), transpose to c-major (bf16)
                    acc_sb = tbuf.tile([128, GPC, C], BF16, tag="accsb")
                    for c in range(GPC):
                        nc.scalar.copy(acc_sb[:, c, :], acc_ts[c][:, 0:C])
                    dwT0 = tbuf.tile([128, 512], BF16, tag="dwT0")
                    dwT1 = tbuf.tile([64, 512], BF16, tag="dwT1")
                    for c in range(GPC):
                        pt = ps_mm.tile([128, 256], BF16, tag="mmb")
                        nc.tensor.transpose(pt[:, 0:128], acc_sb[:, c, 0:128], identb[:, :])
                        nc.tensor.transpose(pt[0:64, 128:256], acc_sb[:, c, 128:192], identb[:, :])
                        nc.vector.tensor_copy(dwT0[:, 128 * c:128 * (c + 1)], pt[:, 0:128])
                        nc.vector.tensor_copy(dwT1[:, 128 * c:128 * (c + 1)], pt[0:64, 128:256])

                    # pointwise
                    for o in range(3):
                        ppw = ps_mm.tile([128, 512], F32, tag="mm")
                        nc.tensor.matmul(
                            ppw[:, :], s_p0[:, 128 * o:128 * (o + 1)], dwT0[:, :],
                            start=True, stop=False,
                        )
                        nc.tensor.matmul(
                            ppw[:, :], s_p1[:, 128 * o:128 * (o + 1)], dwT1[:, :],
                            start=False, stop=True,
                        )
                        osb = obuf.tile([128, 512], BF16, tag="osb")
                        nc.scalar.copy(osb[:, :], ppw[:, :])
                        nc.sync.dma_start(
                            out=out_d[b, 128 * o:128 * (o + 1), 512 * ch:512 * (ch + 1)],
                            in_=osb[:, :],
                        )

    nc.compile()
    _cache["nc"] = nc
    return nc


def _host_prep(x, w_off, b_off, w_dw, w_pw):
    K = 3
    bf = ml_dtypes.bfloat16
    # conv input, zero-padded by 1, c-major
    xcp = np.zeros((B, C, WC, WC), bf)
    xcp[:, :, 1:65, 1:65] = x
    # per-tap dw-scaled gather images: row-pair + x-pair duplicated, px-major
    # xg[b, k, yy*72+xx, (dx*2+r)*C + c] = x[b, c, yy-4+r, xx-4+dx] * dw[c, k]
    wdw = w_dw.reshape(C, K2)
    xg = np.empty((B, K2, NEG, ESZ), bf)
    P2 = np.zeros((B, WG + 1, WG + 1, C), np.float32)
    P2[:, PADG:PADG + H, PADG:PADG + W, :] = np.transpose(x, (0, 2, 3, 1))
    for k in range(K2):
        P2k = (P2 * wdw[None, None, None, :, k]).astype(bf)
        v = xg[:, k].reshape(B, WG, WG, 2, 2, C)
        for dx in range(2):
            for r in range(2):
                v[:, :, :, dx, r, :] = P2k[:, r:r + WG, dx:dx + WG, :]

    # offset conv stationaries, out channels reordered to [y taps | x taps]
    perm = [2 * k for k in range(K2)] + [2 * k + 1 for k in range(K2)]
    wo = np.empty((9, C, 18), np.float32)
    for s in range(9):
        dy, dx = s // 3, s % 3
        wo[s] = w_off[perm, :, dy, dx].T  # [C, 18]
    wo = wo.transpose(1, 0, 2).astype(bf)  # [C, 9, 18]

    # px-major const: pos64 = off + base + ki/kj - 1 + b_off + 64
    i = np.arange(HW)
    hh, ww = i // W, i % W
    cst = np.empty((HW, 18), np.float32)
    for k in range(K2):
        ki, kj = k // K, k % K
        cst[:, k] = hh - 1 + ki + b_off[2 * k] + 64.0
        cst[:, 9 + k] = ww - 1 + kj + b_off[2 * k + 1] + 64.0
    cstT = cst.reshape(NPX, 128, 18).transpose(1, 0, 2).copy()  # [128, NPX, 18]

    wpwT = w_pw.T.astype(bf)  # [C, CO]

    shared = {
        "woff0": wo[:128].copy(),
        "woff1": wo[128:].copy(),
        "cstT": cstT,
        "wpw0": wpwT[:128].copy(),
        "wpw1": wpwT[128:].copy(),
    }
    in_maps = []
    for cid in range(NCORES):
        bs = slice(cid * BPC, (cid + 1) * BPC)
        m = dict(shared)
        m["xc0"] = xcp[bs, :128]
        m["xc1"] = xcp[bs, 128:]
        m["xg"] = xg[bs]
        in_maps.append(m)
    return in_maps


def kernel(x, w_off, b_off, w_dw, w_pw, _trace=False):
    x = np.asarray(x, np.float32)
    w_off = np.asarray(w_off, np.float32)
    b_off = np.asarray(b_off, np.float32)
    w_dw = np.asarray(w_dw, np.float32)
    w_pw = np.asarray(w_pw, np.float32)

    nc = _build()
    in_maps = _host_prep(x, w_off, b_off, w_dw, w_pw)
    res = run_bass_kernel_spmd(nc, in_maps, core_ids=list(range(NCORES)), trace=_trace)
    out = np.concatenate([np.asarray(r["out"], np.float32) for r in res.results], axis=0)
    if _trace:
        kernel.last_exec_ns = res.exec_time_ns
    return out.reshape(B, CO, H, W)
